# Initial kernel scaffold
#
"""AGF layer (softmax-adjacency graph filter) on 8 TRN2 NeuronCores.

Math per (batch b, head h):
  q = x Wq + bq ; k = x Wk + bk ; v = x Wv + bv          (per-head 32-dim slices)
  A = softmax(q k^T / sqrt(32))                           [N, N]
  out_h = sum_k c[h,k] A^k v                              (K_ORDER = 3)
  out = concat_h(out_h) Wo + bo

Sharding: core c handles batch b = c//2 and heads 4*(c%2) .. 4*(c%2)+3
(4 of 8 heads), processed as 2 pairs. Per pair, E^T = exp(S^T) is kept
in SBUF bf16 and the polynomial filter streams it through the PE array
with tile_position-packed skinny matmuls. No max-subtraction in softmax:
|S| <= ~4 for this problem's scale, exp is safe in fp32.

Each core returns two partial projections (one per pair) in transposed
layout [256, N]; the host sums pairs + core-pairs, transposes, adds bo.
"""
import sys
import numpy as np
import ml_dtypes

sys.path.insert(0, "/opt/trn_rl_repo")

import concourse.bass as bass
import concourse.mybir as mybir
from concourse import bacc, tile
from concourse.bass_utils import run_bass_kernel_spmd

BF16 = mybir.dt.bfloat16
F32 = mybir.dt.float32

B, N, D, H, HD, KORD = 4, 2048, 256, 8, 32, 3
NB = N // 128          # 16 m/n blocks of 128
NC4 = N // 512         # 4 chunks of 512
SCALE = 1.0 / np.sqrt(HD)


# ---------------------------------------------------------------- graph ----
def build_graph():
    nc = bacc.Bacc("TRN2", target_bir_lowering=False, debug=False, num_devices=8)

    xT = nc.dram_tensor("xT", [2, 128, N], BF16, kind="ExternalInput")
    WQ = nc.dram_tensor("WQ", [4, 257, 128], BF16, kind="ExternalInput")
    WK = nc.dram_tensor("WK", [4, 257, 128], BF16, kind="ExternalInput")
    WV = nc.dram_tensor("WV", [257, 128], BF16, kind="ExternalInput")
    WO0 = nc.dram_tensor("WO0", [128, 256], BF16, kind="ExternalInput")
    WOK = nc.dram_tensor("WOK", [2, 3, 64, 256], BF16, kind="ExternalInput")
    out_d = nc.dram_tensor("out", [2, 256, N], F32, kind="ExternalOutput")

    recb = nc.dram_tensor("recb", [2, 2, N], F32)          # recip bounce, per pair
    tbounce = nc.dram_tensor("tbounce", [2, 2, 64, N], BF16)  # t_kT bounce, [pair, k-1]
    vbounce = nc.dram_tensor("vbounce", [128, N], BF16)

    from contextlib import ExitStack
    with ExitStack() as ctx, tile.TileContext(nc) as tc:
        wp = ctx.enter_context(tc.tile_pool(name="wp", bufs=1))
        xp = ctx.enter_context(tc.tile_pool(name="xp", bufs=1))
        qkp = ctx.enter_context(tc.tile_pool(name="qkp", bufs=2))
        ep = ctx.enter_context(tc.tile_pool(name="ep", bufs=1))
        vp = ctx.enter_context(tc.tile_pool(name="vp", bufs=1))
        v1p = ctx.enter_context(tc.tile_pool(name="v1p", bufs=2))
        tp = ctx.enter_context(tc.tile_pool(name="tp", bufs=1))
        tnp = ctx.enter_context(tc.tile_pool(name="tnp", bufs=2))
        rp = ctx.enter_context(tc.tile_pool(name="rp", bufs=2))
        rbp = ctx.enter_context(tc.tile_pool(name="rbp", bufs=1))
        op = ctx.enter_context(tc.tile_pool(name="op", bufs=2))
        sps = ctx.enter_context(tc.tile_pool(name="sps", bufs=1, space="PSUM"))
        fps = ctx.enter_context(tc.tile_pool(name="fps", bufs=4, space="PSUM"))

        # ---------------- setup: inputs to SBUF
        xk = []
        for i in range(2):
            t = xp.tile([128, N], BF16, tag=f"xk{i}")
            nc.sync.dma_start(out=t, in_=xT[i])
            xk.append(t)
        ones = wp.tile([1, 512], BF16, tag="ones")
        nc.vector.memset(ones, 1.0)

        wq_t, wk_t = [], []
        for j in range(4):
            tq = wp.tile([257, 128], BF16, tag=f"wq{j}")
            nc.sync.dma_start(out=tq, in_=WQ[j])
            wq_t.append(tq)
            tk = wp.tile([257, 128], BF16, tag=f"wk{j}")
            nc.sync.dma_start(out=tk, in_=WK[j])
            wk_t.append(tk)
        wv_t = wp.tile([257, 128], BF16, tag="wv")
        nc.sync.dma_start(out=wv_t, in_=WV[:, :])
        wo0_t = wp.tile([128, 256], BF16, tag="wo0")
        nc.sync.dma_start(out=wo0_t, in_=WO0[:, :])
        wok_t = {}
        for p in range(2):
            for k in range(3):
                t = wp.tile([64, 256], BF16, tag=f"wok{p}{k}")
                nc.sync.dma_start(out=t, in_=WOK[p, k])
                wok_t[(p, k)] = t

        def project(w_tile, psum, ncs):
            """psum[:, :512] = w_tile.T @ [x; ones] for n-chunk ncs."""
            s = slice(ncs * 512, (ncs + 1) * 512)
            nc.tensor.matmul(psum, w_tile[0:128, :], xk[0][:, s],
                             start=True, stop=False)
            nc.tensor.matmul(psum, w_tile[128:256, :], xk[1][:, s],
                             start=False, stop=False)
            nc.tensor.matmul(psum, w_tile[256:257, :], ones[:, :],
                             start=False, stop=True)

        # vT projection (all 4 local heads) + v-nat via DMA transpose
        vT = vp.tile([128, N], BF16, tag="vT")
        for ncs in range(NC4):
            pv = fps.tile([128, 512], F32, tag="F")
            project(wv_t, pv, ncs)
            nc.vector.tensor_copy(out=vT[:, ncs * 512:(ncs + 1) * 512], in_=pv)
        nc.sync.dma_start(out=vbounce[:, :], in_=vT)
        vnat = vp.tile([128, NB, 128], BF16, tag="vnat")
        nc.sync.dma_start_transpose(out=vnat, in_=vbounce[:, :])

        v1 = []
        for p in range(2):
            t = v1p.tile([128, NB, 66], BF16, tag="v1")
            nc.vector.tensor_copy(out=t[:, :, 0:32], in_=vnat[:, :, 64 * p:64 * p + 32])
            nc.vector.tensor_copy(out=t[:, :, 33:65], in_=vnat[:, :, 64 * p + 32:64 * p + 64])
            nc.vector.memset(t[:, :, 32:33], 1.0)
            nc.vector.memset(t[:, :, 65:66], 1.0)
            v1.append(t)

        # ---------------- per-pair state
        Et = {}     # (p, jh) -> E^T tile  [128, NB, 2048] bf16
        qT = {}     # (p, jh) -> q^T replicated [128, N]
        kT = {}
        tkT = {}    # (p, k) -> t_k^T [64, N] bf16 (k = 1..3)
        tnat = {}   # (p, k) -> natural [128, NB, 64]
        rb = {}     # pair -> rb128 [128, N] f32 broadcast recips

        def gen_qkproj(p):
            for jh in range(2):
                j = 2 * p + jh
                tq = qkp.tile([128, N], BF16, tag="qT")
                tk2 = qkp.tile([128, N], BF16, tag="kT")
                for ncs in range(NC4):
                    pq = fps.tile([128, 512], F32, tag="F")
                    project(wq_t[j], pq, ncs)
                    nc.vector.tensor_copy(out=tq[:, ncs * 512:(ncs + 1) * 512], in_=pq)
                    pk = fps.tile([128, 512], F32, tag="F")
                    project(wk_t[j], pk, ncs)
                    nc.vector.tensor_copy(out=tk2[:, ncs * 512:(ncs + 1) * 512], in_=pk)
                    yield
                qT[(p, jh)] = tq
                kT[(p, jh)] = tk2

        def gen_scores(p):
            """4-way row-packed score matmuls + exp, per (mb, head)."""
            for jh in range(2):
                Et[(p, jh)] = ep.tile([128, NB, N], BF16, tag=f"E{jh}")
            for mb in range(NB):
                for jh in range(2):
                    tq, tk2 = qT[(p, jh)], kT[(p, jh)]
                    ps = sps.tile([128, N], F32, tag="S")
                    for r in range(4):
                        nc.tensor.matmul(
                            ps[:, r * 512:(r + 1) * 512],
                            tk2[32 * r:32 * r + 32, mb * 128:(mb + 1) * 128],
                            tq[32 * r:32 * r + 32, r * 512:(r + 1) * 512],
                            start=True, stop=True, tile_position=(32 * r, 0))
                    nc.scalar.activation(
                        out=Et[(p, jh)][:, mb, :], in_=ps,
                        func=mybir.ActivationFunctionType.Exp)
                    yield

        def gen_s1(p):
            """Filter step 1: u1 = E [v|1]; 2-way col-packed M=33."""
            banks = [fps.tile([128, 512], F32, tag="F") for _ in range(NC4)]
            for mb in range(NB):
                for ncs in range(NC4):
                    st, sp_ = (mb == 0), (mb == NB - 1)
                    s = slice(ncs * 512, (ncs + 1) * 512)
                    for jh in range(2):
                        nc.tensor.matmul(
                            banks[ncs][64 * jh:64 * jh + 33, :],
                            v1[p][:, mb, 33 * jh:33 * jh + 33],
                            Et[(p, jh)][:, mb, s],
                            start=st, stop=sp_, tile_position=(0, 64 * jh))
                yield
            # epilogue: recips -> dram bounce -> broadcast -> scale
            t1 = tp.tile([64, N], BF16, tag="t1T")
            rbt = rbp.tile([128, N], F32, tag="rb")
            for ncs in range(NC4):
                s = slice(ncs * 512, (ncs + 1) * 512)
                rs = rp.tile([2, 512], F32, tag="rec")
                nc.vector.reciprocal(out=rs[0:1, :], in_=banks[ncs][32:33, :])
                nc.vector.reciprocal(out=rs[1:2, :], in_=banks[ncs][96:97, :])
                nc.sync.dma_start(out=recb[p, :, s], in_=rs)
                for jh in range(2):
                    src = recb[p, jh, s]
                    bc = bass.AP(tensor=src.tensor, offset=src.offset,
                                 ap=[[0, 32]] + src.ap)
                    nc.sync.dma_start(out=rbt[64 * jh:64 * jh + 32, s], in_=bc)
                nc.vector.tensor_tensor(out=t1[0:32, s], in0=banks[ncs][0:32, :],
                                        in1=rbt[0:32, s], op=mybir.AluOpType.mult)
                nc.vector.tensor_tensor(out=t1[32:64, s], in0=banks[ncs][64:96, :],
                                        in1=rbt[64:96, s], op=mybir.AluOpType.mult)
                yield
            rb[p] = rbt
            tkT[(p, 1)] = t1
            nc.sync.dma_start(out=tbounce[p, 0], in_=t1)
            tn = tnp.tile([128, NB, 64], BF16, tag="tnat")
            nc.sync.dma_start_transpose(out=tn, in_=tbounce[p, 0])
            tnat[(p, 1)] = tn
            yield

        def gen_s23(p, k):
            """Filter steps 2/3: 4-way col-packed M=32, n-split layout."""
            stat = tnat[(p, k - 1)]
            banks = [fps.tile([128, 512], F32, tag="F") for _ in range(2)]
            for mb in range(NB):
                for cg in range(2):
                    st, sp_ = (mb == 0), (mb == NB - 1)
                    for q4 in range(4):
                        jh = q4 % 2          # head in pair
                        ci = 2 * cg + q4 // 2  # n-chunk
                        nc.tensor.matmul(
                            banks[cg][32 * q4:32 * q4 + 32, :],
                            stat[:, mb, 32 * jh:32 * jh + 32],
                            Et[(p, jh)][:, mb, ci * 512:(ci + 1) * 512],
                            start=st, stop=sp_, tile_position=(0, 32 * q4))
                yield
            tk = tp.tile([64, N], BF16, tag=f"t{k}T")
            rbt = rb[p]
            for cg in range(2):
                for half in range(2):
                    ci = 2 * cg + half
                    s = slice(ci * 512, (ci + 1) * 512)
                    nc.vector.tensor_tensor(
                        out=tk[0:32, s], in0=banks[cg][64 * half:64 * half + 32, :],
                        in1=rbt[0:32, s], op=mybir.AluOpType.mult)
                    nc.vector.tensor_tensor(
                        out=tk[32:64, s], in0=banks[cg][64 * half + 32:64 * half + 64, :],
                        in1=rbt[64:96, s], op=mybir.AluOpType.mult)
                yield
            tkT[(p, k)] = tk
            if k < 3:
                nc.sync.dma_start(out=tbounce[p, k - 1], in_=tk)
                tn = tnp.tile([128, NB, 64], BF16, tag="tnat")
                nc.sync.dma_start_transpose(out=tn, in_=tbounce[p, k - 1])
                tnat[(p, k)] = tn
            yield

        def gen_proj(p):
            for mc in range(2):
                cs = slice(mc * 128, (mc + 1) * 128)
                for ncs in range(NC4):
                    s = slice(ncs * 512, (ncs + 1) * 512)
                    pp = fps.tile([128, 512], F32, tag="F")
                    first = True
                    if p == 0:
                        nc.tensor.matmul(pp, wo0_t[:, cs], vT[:, s],
                                         start=True, stop=False)
                        first = False
                    for k in range(1, 4):
                        nc.tensor.matmul(pp, wok_t[(p, k - 1)][:, cs],
                                         tkT[(p, k)][:, s],
                                         start=first, stop=(k == 3))
                        first = False
                    ost = op.tile([128, 512], F32, tag="ost")
                    nc.vector.tensor_copy(out=ost, in_=pp)
                    nc.sync.dma_start(out=out_d[p, cs, s], in_=ost)
                    yield

        # ---------------- drive emission with interleave
        def drain(g):
            for _ in g:
                pass

        def interleave(main, bg, ratio):
            """Advance main; after each main quantum, advance bg `ratio` times."""
            for _ in main:
                for _ in range(ratio):
                    if bg is not None:
                        try:
                            next(bg)
                        except StopIteration:
                            bg = None
            # main done; finish bg
            if bg is not None:
                drain(bg)

        def chain(*gens):
            for g in gens:
                yield from g

        def gen_phaseA(p):
            """scores interleaved with s1 (s1 trails scores by one mb wave)."""
            yield from gen_qkproj(p)
            sc = gen_scores(p)
            s1 = gen_s1(p)
            # emit 2 score groups (one mb, both heads), then 1 s1 wave
            done_sc = False
            while not done_sc:
                for _ in range(2):
                    try:
                        next(sc)
                    except StopIteration:
                        done_sc = True
                        break
                    yield
                try:
                    next(s1)
                    yield
                except StopIteration:
                    s1 = None
            if s1 is not None:
                for _ in s1:
                    yield

        def gen_phaseB(p):
            yield from gen_s23(p, 2)
            yield from gen_s23(p, 3)
            yield from gen_proj(p)

        drain(gen_phaseA(0))
        interleave(gen_phaseA(1), gen_phaseB(0), ratio=1)
        drain(gen_phaseB(1))

    nc.compile()
    return nc


_graph_cache = None


def _get_graph():
    global _graph_cache
    if _graph_cache is None:
        _graph_cache = build_graph()
    return _graph_cache


# ---------------------------------------------------------------- host ----
def _prep_core_inputs(c, x, Wq, bq, Wk, bk, Wv, bv, Wo, coeffs):
    bf = ml_dtypes.bfloat16
    b, hh = c // 2, c % 2
    heads = [4 * hh + j for j in range(4)]

    xTb = np.ascontiguousarray(x[b].T.astype(bf)).reshape(2, 128, N)

    def aug_rep(W, bias, h, scale):
        cols = slice(h * HD, (h + 1) * HD)
        wrep = np.tile(W[:, cols] * scale, (1, 4))            # [256, 128]
        brep = np.tile(bias[cols] * scale, 4)[None, :]        # [1, 128]
        return np.concatenate([wrep, brep], 0).astype(bf)     # [257, 128]

    WQc = np.stack([aug_rep(Wq, bq, h, SCALE) for h in heads])
    WKc = np.stack([aug_rep(Wk, bk, h, 1.0) for h in heads])

    wv_cols = np.concatenate([Wv[:, h * HD:(h + 1) * HD] for h in heads], 1)
    bv_cols = np.concatenate([bv[h * HD:(h + 1) * HD] for h in heads])[None, :]
    WVc = np.concatenate([wv_cols, bv_cols], 0).astype(bf)    # [257, 128]

    wo_rows = np.concatenate([Wo[h * HD:(h + 1) * HD, :] for h in heads], 0)  # [128, 256]
    c0 = np.concatenate([np.full(HD, coeffs[h, 0]) for h in heads])
    WO0c = (wo_rows * c0[:, None]).astype(bf)
    WOKc = np.zeros((2, 3, 64, 256), bf)
    for p in range(2):
        rows = wo_rows[64 * p:64 * p + 64]
        for k in range(1, 4):
            ck = np.concatenate([np.full(HD, coeffs[heads[2 * p], k]),
                                 np.full(HD, coeffs[heads[2 * p + 1], k])])
            WOKc[p, k - 1] = (rows * ck[:, None]).astype(bf)

    return {"xT": xTb, "WQ": WQc, "WK": WKc, "WV": WVc,
            "WO0": WO0c, "WOK": WOKc}


def kernel(**inputs):
    x = np.asarray(inputs["x"], np.float32)
    Wq, bq = np.asarray(inputs["Wq"], np.float32), np.asarray(inputs["bq"], np.float32)
    Wk, bk = np.asarray(inputs["Wk"], np.float32), np.asarray(inputs["bk"], np.float32)
    Wv, bv = np.asarray(inputs["Wv"], np.float32), np.asarray(inputs["bv"], np.float32)
    Wo, bo = np.asarray(inputs["Wo"], np.float32), np.asarray(inputs["bo"], np.float32)
    coeffs = np.asarray(inputs["coeffs"], np.float32)

    nc = _get_graph()
    in_maps = [_prep_core_inputs(c, x, Wq, bq, Wk, bk, Wv, bv, Wo, coeffs)
               for c in range(8)]
    res = run_bass_kernel_spmd(nc, in_maps, core_ids=list(range(8))).results

    out = np.zeros((B, N, D), np.float32)
    for c in range(8):
        o = res[c]["out"]                     # [2, 256, N]
        out[c // 2] += (o[0] + o[1]).T
    out += bo[None, None, :]
    return out


# revision 8
# speedup vs baseline: 1.3460x; 1.3460x over previous
"""AGF layer (softmax-adjacency graph filter) on 8 TRN2 NeuronCores.

Math per (batch b, head h):
  q = x Wq + bq ; k = x Wk + bk ; v = x Wv + bv          (per-head 32-dim slices)
  A = softmax(q k^T / sqrt(32))                           [N, N]
  out_h = sum_k c[h,k] A^k v                              (K_ORDER = 3)
  out = concat_h(out_h) Wo + bo

Sharding: core c handles batch b = c//2 and heads 4*(c%2) .. 4*(c%2)+3,
processed as 2 pairs. Per pair, E^T = exp(S^T) lives in SBUF bf16; the
polynomial filter streams it through the PE with tile_position-packed
skinny matmuls. Softmax denominators come from a fused ones-column in
the step-1 stationary ([v|1]); no max-subtraction (|S| <= ~4 here).

Each core outputs two partial projections (one per pair) in transposed
layout [256, N]; host sums pairs + core-pairs, transposes, adds bo.
"""
import sys
import numpy as np
import ml_dtypes

sys.path.insert(0, "/opt/trn_rl_repo")

import concourse.bass as bass
import concourse.mybir as mybir
from concourse import bacc, tile
from concourse.bass_utils import run_bass_kernel_spmd

BF16 = mybir.dt.bfloat16
F32 = mybir.dt.float32

B, N, D, H, HD, KORD = 4, 2048, 256, 8, 32, 3
NB = N // 128          # 16 blocks of 128
NC4 = N // 512         # 4 chunks of 512
SCALE = 1.0 / np.sqrt(HD)


# ---------------------------------------------------------------- graph ----
def build_graph(reps=1, hw_loop=0):
    nc = bacc.Bacc("TRN2", target_bir_lowering=False, debug=False, num_devices=8)

    xT = nc.dram_tensor("xT", [2, 128, N], BF16, kind="ExternalInput")
    WQ = nc.dram_tensor("WQ", [4, 257, 128], BF16, kind="ExternalInput")
    WK = nc.dram_tensor("WK", [4, 257, 128], BF16, kind="ExternalInput")
    WV = nc.dram_tensor("WV", [257, 128], BF16, kind="ExternalInput")
    WO0 = nc.dram_tensor("WO0", [128, 256], BF16, kind="ExternalInput")
    WOK = nc.dram_tensor("WOK", [2, 3, 64, 256], BF16, kind="ExternalInput")
    out_d = nc.dram_tensor("out", [2, 256, N], F32, kind="ExternalOutput")

    recb = nc.dram_tensor("recb", [2, 2, N], BF16)             # recip bounce
    tbounce = nc.dram_tensor("tbounce", [2, 2, 64, N], BF16)  # [pair, k-1]
    vbounce = nc.dram_tensor("vbounce", [128, N], BF16)

    from contextlib import ExitStack
    with tile.TileContext(nc) as tc, ExitStack() as ctx:
        wp = ctx.enter_context(tc.tile_pool(name="wp", bufs=1))
        xp = ctx.enter_context(tc.tile_pool(name="xp", bufs=1))
        qkp = ctx.enter_context(tc.tile_pool(name="qkp", bufs=2))
        ep = ctx.enter_context(tc.tile_pool(name="ep", bufs=1))
        vp = ctx.enter_context(tc.tile_pool(name="vp", bufs=1))
        v1p = ctx.enter_context(tc.tile_pool(name="v1p", bufs=2))
        tp = ctx.enter_context(tc.tile_pool(name="tp", bufs=1))
        tnp = ctx.enter_context(tc.tile_pool(name="tnp", bufs=2))
        rp = ctx.enter_context(tc.tile_pool(name="rp", bufs=1))
        rbp = ctx.enter_context(tc.tile_pool(name="rbp", bufs=1))
        op = ctx.enter_context(tc.tile_pool(name="op", bufs=2))
        sps = ctx.enter_context(tc.tile_pool(name="sps", bufs=1, space="PSUM"))
        fps = ctx.enter_context(tc.tile_pool(name="fps", bufs=4, space="PSUM"))

        # ---------------- setup: inputs to SBUF
        xk = []
        for i in range(2):
            t = xp.tile([128, N], BF16, tag=f"xk{i}")
            nc.sync.dma_start(out=t, in_=xT[i])
            xk.append(t)
        ones = wp.tile([1, 512], BF16, tag="ones")
        nc.vector.memset(ones, 1.0)

        wq_t, wk_t = [], []
        for j in range(4):
            tq = wp.tile([128, 2, 128], BF16, tag=f"wq{j}")
            nc.sync.dma_start(out=tq, in_=WQ[j, 0:256, :].rearrange(
                "(kb p) m -> p kb m", p=128))
            wq_t.append(tq)
            tk = wp.tile([128, 2, 128], BF16, tag=f"wk{j}")
            nc.sync.dma_start(out=tk, in_=WK[j, 0:256, :].rearrange(
                "(kb p) m -> p kb m", p=128))
            wk_t.append(tk)
        wqb = wp.tile([1, 4, 128], BF16, tag="wqb")
        nc.sync.dma_start(out=wqb, in_=WQ[:, 256:257, :].rearrange("j o m -> o j m"))
        wkb = wp.tile([1, 4, 128], BF16, tag="wkb")
        nc.sync.dma_start(out=wkb, in_=WK[:, 256:257, :].rearrange("j o m -> o j m"))

        wv_t = wp.tile([128, 2, 128], BF16, tag="wv")
        nc.sync.dma_start(out=wv_t, in_=WV[0:256, :].rearrange(
            "(kb p) m -> p kb m", p=128))
        wvb = wp.tile([1, 128], BF16, tag="wvb")
        nc.sync.dma_start(out=wvb, in_=WV[256:257, :])
        wo0_t = wp.tile([128, 256], BF16, tag="wo0")
        nc.sync.dma_start(out=wo0_t, in_=WO0[:, :])
        wok_t = {}
        for p in range(2):
            for k in range(3):
                t = wp.tile([64, 256], BF16, tag=f"wok{p}{k}")
                nc.sync.dma_start(out=t, in_=WOK[p, k])
                wok_t[(p, k)] = t

        def project(w_tile, bias_ap, psum, ncs):
            """psum = w.T @ x^T[:, chunk] + bias (via augmented ones row)."""
            s = slice(ncs * 512, (ncs + 1) * 512)
            nc.tensor.matmul(psum, w_tile[:, 0, :], xk[0][:, s],
                             start=True, stop=False)
            nc.tensor.matmul(psum, w_tile[:, 1, :], xk[1][:, s],
                             start=False, stop=False)
            nc.tensor.matmul(psum, bias_ap, ones[:, :],
                             start=False, stop=True)

        # vT projection (all 4 local heads) + v-nat via DMA transpose
        vT = vp.tile([128, N], BF16, tag="vT")
        for ncs in range(NC4):
            pv = fps.tile([128, 512], F32, tag="F")
            project(wv_t, wvb[:, :], pv, ncs)
            nc.vector.tensor_copy(out=vT[:, ncs * 512:(ncs + 1) * 512], in_=pv)
        nc.sync.dma_start(out=vbounce[:, :], in_=vT)
        vnat = vp.tile([128, NB, 128], BF16, tag="vnat")
        nc.sync.dma_start_transpose(out=vnat, in_=vbounce[:, :])

        v1 = []
        for p in range(2):
            t = v1p.tile([128, NB, 66], BF16, tag="v1")
            nc.vector.tensor_copy(out=t[:, :, 0:32], in_=vnat[:, :, 64 * p:64 * p + 32])
            nc.vector.tensor_copy(out=t[:, :, 33:65],
                                  in_=vnat[:, :, 64 * p + 32:64 * p + 64])
            nc.vector.memset(t[:, :, 32:33], 1.0)
            nc.vector.memset(t[:, :, 65:66], 1.0)
            v1.append(t)

        # ---------------- per-pair state
        Et = {}
        qT = {}
        kT = {}
        tkT = {}
        tnat = {}
        rb = {}

        def gen_qkproj(p):
            for jh in range(2):
                j = 2 * p + jh
                tq = qkp.tile([128, N], BF16, tag="qT")
                tk2 = qkp.tile([128, N], BF16, tag="kT")
                for ncs in range(NC4):
                    pq = fps.tile([128, 512], F32, tag="F")
                    project(wq_t[j], wqb[:, j, :], pq, ncs)
                    nc.vector.tensor_copy(out=tq[:, ncs * 512:(ncs + 1) * 512], in_=pq)
                    yield
                    pk = fps.tile([128, 512], F32, tag="F")
                    project(wk_t[j], wkb[:, j, :], pk, ncs)
                    nc.vector.tensor_copy(out=tk2[:, ncs * 512:(ncs + 1) * 512], in_=pk)
                    yield
                qT[(p, jh)] = tq
                kT[(p, jh)] = tk2

        def emit_score_group(p, jh, mb):
            tq, tk2 = qT[(p, jh)], kT[(p, jh)]
            ps = sps.tile([128, N], F32, tag="S")
            for r in range(4):
                nc.tensor.matmul(
                    ps[:, r * 512:(r + 1) * 512],
                    tk2[32 * r:32 * r + 32, mb * 128:(mb + 1) * 128],
                    tq[32 * r:32 * r + 32, r * 512:(r + 1) * 512],
                    start=True, stop=True, tile_position=(32 * r, 0))
            nc.scalar.activation(out=Et[(p, jh)][:, mb, :], in_=ps,
                                 func=mybir.ActivationFunctionType.Exp)

        def gen_scores_s1(p):
            """Scores (ACT-paced) with s1 waves trailing one mb behind."""
            for jh in range(2):
                Et[(p, jh)] = ep.tile([128, NB, N], BF16, tag=f"E{jh}", name=f"E_{p}_{jh}")
            banks = [fps.tile([128, 512], F32, tag="F", name=f"s1b{i}") for i in range(NC4)]

            def s1_wave(mb):
                for ncs in range(NC4):
                    st, sp_ = (mb == 0), (mb == NB - 1)
                    for jh in range(2):
                        nc.tensor.matmul(
                            banks[ncs][64 * jh:64 * jh + 33, :],
                            v1[p][:, mb, 33 * jh:33 * jh + 33],
                            Et[(p, jh)][:, mb, ncs * 512:(ncs + 1) * 512],
                            start=st, stop=sp_, tile_position=(0, 64 * jh))

            for mb in range(NB):
                emit_score_group(p, 0, mb)
                yield
                emit_score_group(p, 1, mb)
                yield
                s1_wave(mb)
                yield

            # epilogue: recip -> dram bounce -> broadcast -> scale -> t1T
            t1 = tp.tile([64, N], BF16, tag="t1T")
            rbt = rbp.tile([128, N], BF16, tag="rb")
            for ncs in range(NC4):
                s = slice(ncs * 512, (ncs + 1) * 512)
                rsa = rp.tile([1, 512], F32, tag="reca")
                rsb = rp.tile([1, 512], F32, tag="recb")
                nc.vector.reciprocal(out=rsa, in_=banks[ncs][32:33, :])
                nc.vector.reciprocal(out=rsb, in_=banks[ncs][96:97, :])
                rca = rp.tile([1, 512], BF16, tag="reca16")
                rcb = rp.tile([1, 512], BF16, tag="recb16")
                nc.vector.tensor_copy(out=rca, in_=rsa)
                nc.vector.tensor_copy(out=rcb, in_=rsb)
                nc.sync.dma_start(out=recb[p, 0, s], in_=rca)
                nc.sync.dma_start(out=recb[p, 1, s], in_=rcb)
                for jh in range(2):
                    src = recb[p, jh, s]
                    bc = bass.AP(tensor=src.tensor, offset=src.offset,
                                 ap=[[0, 32]] + src.ap)
                    nc.sync.dma_start(out=rbt[64 * jh:64 * jh + 32, s], in_=bc)
                nc.vector.tensor_tensor(out=t1[0:32, s], in0=banks[ncs][0:32, :],
                                        in1=rbt[0:32, s], op=mybir.AluOpType.mult)
                nc.vector.tensor_tensor(out=t1[32:64, s], in0=banks[ncs][64:96, :],
                                        in1=rbt[64:96, s], op=mybir.AluOpType.mult)
                yield
            rb[p] = rbt
            tkT[(p, 1)] = t1
            nc.sync.dma_start(out=tbounce[p, 0], in_=t1)
            tn = tnp.tile([128, NB, 64], BF16, tag="tnat")
            nc.sync.dma_start_transpose(out=tn, in_=tbounce[p, 0])
            tnat[(p, 1)] = tn
            yield

        def gen_s23(p, k):
            """Filter steps 2/3: 4-way col-packed M=32, n-split psum layout."""
            stat = tnat[(p, k - 1)]
            banks = [fps.tile([128, 512], F32, tag="F", name=f"s{k}b{i}") for i in range(2)]
            for mb in range(NB):
                for cg in range(2):
                    st, sp_ = (mb == 0), (mb == NB - 1)
                    for q4 in range(4):
                        jh = q4 % 2
                        ci = 2 * cg + q4 // 2
                        nc.tensor.matmul(
                            banks[cg][32 * q4:32 * q4 + 32, :],
                            stat[:, mb, 32 * jh:32 * jh + 32],
                            Et[(p, jh)][:, mb, ci * 512:(ci + 1) * 512],
                            start=st, stop=sp_, tile_position=(0, 32 * q4))
                yield
            tk = tp.tile([64, N], BF16, tag=f"t{k}T")
            rbt = rb[p]
            for cg in range(2):
                for half in range(2):
                    ci = 2 * cg + half
                    s = slice(ci * 512, (ci + 1) * 512)
                    nc.vector.tensor_tensor(
                        out=tk[0:32, s], in0=banks[cg][64 * half:64 * half + 32, :],
                        in1=rbt[0:32, s], op=mybir.AluOpType.mult)
                    nc.vector.tensor_tensor(
                        out=tk[32:64, s],
                        in0=banks[cg][64 * half + 32:64 * half + 64, :],
                        in1=rbt[64:96, s], op=mybir.AluOpType.mult)
                yield
            tkT[(p, k)] = tk
            if k < 3:
                nc.sync.dma_start(out=tbounce[p, k - 1], in_=tk)
                tn = tnp.tile([128, NB, 64], BF16, tag="tnat")
                nc.sync.dma_start_transpose(out=tn, in_=tbounce[p, k - 1])
                tnat[(p, k)] = tn
            yield

        def gen_proj(p):
            for mc in range(2):
                cs = slice(mc * 128, (mc + 1) * 128)
                for ncs in range(NC4):
                    s = slice(ncs * 512, (ncs + 1) * 512)
                    pp = fps.tile([128, 512], F32, tag="F")
                    first = True
                    if p == 0:
                        nc.tensor.matmul(pp, wo0_t[:, cs], vT[:, s],
                                         start=True, stop=False)
                        first = False
                    for k in range(1, 4):
                        nc.tensor.matmul(pp, wok_t[(p, k - 1)][:, cs],
                                         tkT[(p, k)][:, s],
                                         start=first, stop=(k == 3))
                        first = False
                    ost = op.tile([128, 512], F32, tag="ost")
                    nc.vector.tensor_copy(out=ost, in_=pp)
                    nc.sync.dma_start(out=out_d[p, cs, s], in_=ost)
                    yield

        # ---------------- emission schedule
        def drain(g):
            for _ in g:
                pass

        def alternate(a, b):
            """Interleave two generators quantum by quantum."""
            while True:
                done = 0
                for g in (a, b):
                    if g is not None:
                        try:
                            next(g)
                        except StopIteration:
                            done += 1
                    else:
                        done += 1
                if done == 2:
                    break

        def emit_body():
            drain(gen_qkproj(0))
            drain(gen_scores_s1(0))
            drain(gen_s23(0, 2))
            drain(gen_s23(0, 3))
            alternate(gen_qkproj(1), gen_proj(0))
            drain(gen_scores_s1(1))
            drain(gen_s23(1, 2))
            drain(gen_s23(1, 3))
            drain(gen_proj(1))

        if hw_loop:
            with tc.For_i(0, hw_loop, 1) as _i:
                emit_body()
        else:
            for _rep in range(reps):
                emit_body()

    nc.compile()
    return nc


_graph_cache = None


def _get_graph():
    global _graph_cache
    if _graph_cache is None:
        _graph_cache = build_graph()
    return _graph_cache


# ---------------------------------------------------------------- host ----
def _prep_core_inputs(c, x, Wq, bq, Wk, bk, Wv, bv, Wo, coeffs):
    bf = ml_dtypes.bfloat16
    b, hh = c // 2, c % 2
    heads = [4 * hh + j for j in range(4)]

    xTb = np.ascontiguousarray(x[b].T.astype(bf)).reshape(2, 128, N)

    def aug_rep(W, bias, h, scale):
        cols = slice(h * HD, (h + 1) * HD)
        wrep = np.tile(W[:, cols] * scale, (1, 4))            # [256, 128]
        brep = np.tile(bias[cols] * scale, 4)[None, :]        # [1, 128]
        return np.concatenate([wrep, brep], 0).astype(bf)     # [257, 128]

    WQc = np.stack([aug_rep(Wq, bq, h, SCALE) for h in heads])
    WKc = np.stack([aug_rep(Wk, bk, h, 1.0) for h in heads])

    wv_cols = np.concatenate([Wv[:, h * HD:(h + 1) * HD] for h in heads], 1)
    bv_cols = np.concatenate([bv[h * HD:(h + 1) * HD] for h in heads])[None, :]
    WVc = np.concatenate([wv_cols, bv_cols], 0).astype(bf)    # [257, 128]

    wo_rows = np.concatenate([Wo[h * HD:(h + 1) * HD, :] for h in heads], 0)
    c0 = np.concatenate([np.full(HD, coeffs[h, 0]) for h in heads])
    WO0c = (wo_rows * c0[:, None]).astype(bf)
    WOKc = np.zeros((2, 3, 64, 256), bf)
    for p in range(2):
        rows = wo_rows[64 * p:64 * p + 64]
        for k in range(1, 4):
            ck = np.concatenate([np.full(HD, coeffs[heads[2 * p], k]),
                                 np.full(HD, coeffs[heads[2 * p + 1], k])])
            WOKc[p, k - 1] = (rows * ck[:, None]).astype(bf)

    return {"xT": xTb, "WQ": WQc, "WK": WKc, "WV": WVc,
            "WO0": WO0c, "WOK": WOKc}


def kernel(**inputs):
    x = np.asarray(inputs["x"], np.float32)
    Wq, bq = np.asarray(inputs["Wq"], np.float32), np.asarray(inputs["bq"], np.float32)
    Wk, bk = np.asarray(inputs["Wk"], np.float32), np.asarray(inputs["bk"], np.float32)
    Wv, bv = np.asarray(inputs["Wv"], np.float32), np.asarray(inputs["bv"], np.float32)
    Wo, bo = np.asarray(inputs["Wo"], np.float32), np.asarray(inputs["bo"], np.float32)
    coeffs = np.asarray(inputs["coeffs"], np.float32)

    nc = _get_graph()
    in_maps = [_prep_core_inputs(c, x, Wq, bq, Wk, bk, Wv, bv, Wo, coeffs)
               for c in range(8)]
    res = run_bass_kernel_spmd(nc, in_maps, core_ids=list(range(8))).results

    out = np.zeros((B, N, D), np.float32)
    for c in range(8):
        o = res[c]["out"]                     # [2, 256, N]
        out[c // 2] += (o[0] + o[1]).T
    out += bo[None, None, :]
    return out
